# revision 31
# baseline (speedup 1.0000x reference)
"""ComplexMoE Trainium2 kernel.

Computes, for z_real/z_imag [N, D], gate weights Wg [E, 2D], bg [E], and
per-expert complex weights Wr/Wi [E, D, D]:

    gates = softmax(concat(z_r, z_i) @ Wg.T + bg)            [N, E]
    out_r = sum_e gates[:, e] * (z_r @ Wr_e.T - z_i @ Wi_e.T)
    out_i = sum_e gates[:, e] * (z_i @ Wr_e.T + z_r @ Wi_e.T)
    return stack([out_r, out_i])                             [2, N, D]

Strategy: data-parallel over tokens across 8 NeuronCores (1024 tokens each,
gate + expert weights replicated).  Per expert the complex matmul uses the
3-multiplication Karatsuba form:

    P1 = z_r @ Wr_e.T;  P2 = z_i @ Wi_e.T;  P3 = (z_r+z_i) @ (Wr_e+Wi_e).T
    out_r_e = P1 - P2;  out_i_e = P3 - P1 - P2

All 1024 tokens stay resident in SBUF (bf16), so each expert weight slice
streams from HBM exactly once.  Matmul operands are bf16 (same PE rate as
fp32r, half the HBM traffic, and FWL weight loads); PSUM accumulation and
the gated expert accumulators are fp32, so the only precision loss is bf16
operand rounding (measured ~4.1e-3 absmax rel on the reference
distribution, vs the 2e-2 gate).  Matmuls keep tokens on PSUM partitions
(stationary = z^T chunks, moving = W^T) so the per-token gate is a
per-partition scalar applied by fused scalar_tensor_tensor ops into three
SBUF accumulators; the final combines ride the Pool engine inline with the
last expert pass.

Perf notes (measured on trn2 via ntff profiles):
 - The PE streams 1 moving column/cycle at 2.4 GHz -> 3072 [128x128]x
   [128x512] matmuls = 665 us/core is the hard tensor-engine floor.
 - Each matmul normally pays a partially-exposed LDWEIGHTS (~45 ns);
   sharing one stationary z chunk across EPG=4 consecutive matmuls into 4
   PSUM banks (experts e..e+3 of the same product) removes nearly all of
   it (808 -> 675 us PE-active).
 - z loads, half of each weight group, and output stores ride the ACT
   hardware DMA queue; the rest rides the SP queue.  The two queues
   genuinely run in parallel (~350 GB/s each), halving the 8 MB critical
   startup prefix.
 - 48 throwaway warm-up matmuls keep the PE HAM clock-gate at K=8/8
   through the startup DMA window.
"""

import sys

try:
    import concourse.bass as bass  # noqa: F401
except ImportError:
    sys.path.insert(0, "/opt/trn_rl_repo")

import numpy as np
import ml_dtypes

import concourse.bass as bass
from concourse import bacc
import concourse.mybir as mybir
from concourse.tile import TileContext
from concourse.bass_utils import run_bass_kernel_spmd

dt = mybir.dt
BF16 = ml_dtypes.bfloat16

# ---------------------------------------------------------------- config
N_CORES = 8
N = 8192
D = 1024
E = 8
N_LOC = N // N_CORES            # tokens per core
P = 128                         # partitions
IC = D // P                     # 8 contraction chunks per z tensor
OBLK = 512                      # output-feature block (one PSUM bank)
N_OBLK = D // OBLK              # 2
TCH = N_LOC // P                # 8 token chunks of 128
GTG = 2                         # gate token groups (PSUM free-size limit)
GT = N_LOC // GTG               # 512 tokens per gate group

EPG = 4                         # experts per group (stationary z chunk shared
                                # across EPG consecutive matmuls -> 1 LDWEIGHTS
                                # per EPG matmuls if codegen elides duplicates)
N_EG = E // EPG                 # 2

TRACE = False                   # set by test harness to capture HW timing
LAST_RESULTS = None             # BassKernelResults of the last run

_BUILT = None


def _build_module():
    nc = bacc.Bacc("TRN2", target_bir_lowering=False, debug=False)

    # Drop the (unused) software-DGE queue declaration: this kernel only
    # issues hardware-DGE transfers (sync/scalar), and each declared ring
    # costs a completion semaphore that the NEFF epilogue clears serially
    # (~110 ns x 16 rings on every engine at exit).
    nc.m.queues = [q for q in nc.m.queues if getattr(q, "is_HWDGE", False)]

    bf = dt.bfloat16
    d_zr = nc.dram_tensor("zr3", [P, IC, N_LOC], bf, kind="ExternalInput").ap()
    d_zi = nc.dram_tensor("zi3", [P, IC, N_LOC], bf, kind="ExternalInput").ap()
    WSHP = [N_OBLK * E, P, IC, OBLK]
    d_wr = nc.dram_tensor("wr4", WSHP, bf, kind="ExternalInput").ap()
    d_wi = nc.dram_tensor("wi4", WSHP, bf, kind="ExternalInput").ap()
    d_ws = nc.dram_tensor("ws4", WSHP, bf, kind="ExternalInput").ap()
    d_wg = nc.dram_tensor("wg3", [P, 2 * IC, E], bf, kind="ExternalInput").ap()
    d_bg = nc.dram_tensor("bgc", [E, 1], dt.float32, kind="ExternalInput").ap()
    OSHP = [N_OBLK * TCH, P, OBLK]
    d_or = nc.dram_tensor("outr", OSHP, dt.float32, kind="ExternalOutput").ap()
    d_oi = nc.dram_tensor("outi", OSHP, dt.float32, kind="ExternalOutput").ap()

    AF = mybir.ActivationFunctionType
    ALU = mybir.AluOpType

    with TileContext(nc, trace_sim=False) as tc:
        with (
            tc.tile_pool(name="cst", bufs=1) as cpool,
            tc.tile_pool(name="zres", bufs=1) as zpool,
            tc.tile_pool(name="wmov", bufs=2) as wpool,
            tc.tile_pool(name="accs", bufs=1) as apool,
            tc.tile_pool(name="gate", bufs=2) as gpool,
            tc.tile_pool(name="outs", bufs=2) as opool,
            tc.tile_pool(name="ps_main", bufs=2, space="PSUM") as pspool,
        ):
            # ---- constants: gate weights + bias
            wg = cpool.tile([P, 2 * IC, E], bf, name="wg")
            nc.sync.dma_start(out=wg[:], in_=d_wg)
            bgc = cpool.tile([E, 1], dt.float32, name="bgc")
            nc.sync.dma_start(out=bgc[:], in_=d_bg)
            ident = cpool.tile([E, E], dt.float32, name="ident")
            from concourse.masks import make_identity

            make_identity(nc, ident[:])

            # ---- resident z^T tensors (full 1024 tokens): [P, IC, N_LOC]
            # z rides the ACT hardware DMA queue so it loads in parallel with
            # the first weight tiles on the SP queue; zs = zr + zi is computed
            # on-device (DVE) -- it is not needed until the third product pass.
            zr = zpool.tile([P, IC, N_LOC], bf, name="zr", tag="zr")
            zi = zpool.tile([P, IC, N_LOC], bf, name="zi", tag="zi")
            zs = zpool.tile([P, IC, N_LOC], bf, name="zs", tag="zs")
            nc.sync.dma_start(out=zr[:], in_=d_zr)
            nc.scalar.dma_start(out=zi[:], in_=d_zi)

            # ---- PE warm-up: throwaway matmuls on the (tiny, already
            # loaded) gate weights keep the PE HAM clock-gate engaged
            # through the z/weight DMA window, so the gate matmuls and
            # first expert chains run at full clock
            for wu in range(48):
                wups = pspool.tile([E, P], dt.float32, name="wup", tag="pj3")
                nc.tensor.matmul(
                    wups[:], lhsT=wg[:, 0, :], rhs=wg[:, :, :], start=True, stop=True
                )

            # ---- gates: logits^T [E, GT] in PSUM, softmax via ACT + DVE
            # (gate PSUM tiles borrow the main pool's expert tags pj0/pj1 --
            # they are drained long before the main loop's allocations rotate
            # around to the same slots)
            g_all = gpool.tile([P, TCH, E], dt.float32, name="g_all", tag="g_all")
            for tg in range(GTG):
                tsl = bass.ts(tg, GT)
                lgT = pspool.tile([E, GT], dt.float32, name="lgT", tag="pj0")
                for c in range(2 * IC):
                    zsrc = zr if c < IC else zi
                    nc.tensor.matmul(
                        lgT[:],
                        lhsT=wg[:, c, :],
                        rhs=zsrc[:, c % IC, tsl],
                        start=(c == 0),
                        stop=(c == 2 * IC - 1),
                    )
                uT = gpool.tile([E, GT], dt.float32, name="uT", tag="uT")
                nc.scalar.activation(uT[:], lgT[:], AF.Exp, bias=bgc[:])
                for t in range(GT // P):
                    gt = tg * (GT // P) + t
                    tp = pspool.tile([P, E], dt.float32, name="tp", tag="pj1")
                    nc.tensor.transpose(
                        tp[:], in_=uT[:, bass.ts(t, P)], identity=ident[:]
                    )
                    s = gpool.tile([P, 1], dt.float32, name="s", tag="s")
                    nc.vector.tensor_reduce(
                        s[:], tp[:], axis=mybir.AxisListType.X, op=ALU.add
                    )
                    r = gpool.tile([P, 1], dt.float32, name="r", tag="r")
                    nc.vector.reciprocal(r[:], s[:])
                    nc.vector.tensor_scalar_mul(g_all[:, gt, :], tp[:], r[:])

            # zs = zr + zi on the Pool engine: on DVE (wherever the
            # scheduler places it) the 4.4 us add delays the first PSUM
            # drains and stalls the PE's bank rotation during startup.
            # GpSimd is idle until the first o_r combines ~100 us in, and zs
            # is not consumed until the third product pass even later.
            nc.gpsimd.tensor_add(out=zs[:], in0=zr[:], in1=zi[:])

            # ---- main loop: weights stream once; per-product passes with
            # EPG experts sharing each stationary z chunk (one LDWEIGHTS
            # feeds EPG matmuls into EPG PSUM banks)
            for ob in range(N_OBLK):
                acc1 = apool.tile([P, TCH, OBLK], dt.float32, name="acc1", tag="acc1")
                acc2 = apool.tile([P, TCH, OBLK], dt.float32, name="acc2", tag="acc2")
                acc3 = apool.tile([P, TCH, OBLK], dt.float32, name="acc3", tag="acc3")
                acc12 = apool.tile(
                    [P, TCH, OBLK], dt.float32, name="acc12", tag="acc12"
                )
                for p_i, (zt, d_w, acc) in enumerate(
                    ((zr, d_wr, acc1), (zi, d_wi, acc2), (zs, d_ws, acc3))
                ):
                    for eg in range(N_EG):
                        wts = []
                        for j in range(EPG):
                            e = eg * EPG + j
                            wt = wpool.tile(
                                [P, IC, OBLK], bf, name=f"w{j}", tag=f"w{j}"
                            )
                            dma_eng = nc.sync if j < EPG // 2 else nc.scalar
                            dma_eng.dma_start(out=wt[:], in_=d_w[ob * E + e])
                            wts.append(wt)

                        for t in range(TCH):
                            pks = [
                                pspool.tile(
                                    [P, OBLK], dt.float32, name=f"pj{j}", tag=f"pj{j}"
                                )
                                for j in range(EPG)
                            ]
                            # The very last chain group runs expert-major
                            # (full c-chain per expert) so each bank's STT
                            # can start as soon as that expert's chain ends;
                            # only the final expert's STT + store trail the
                            # last matmul (~0.75 us) instead of the whole
                            # 4-STT chain (~3 us).  Interleaving the STT
                            # emission per-j below relies on the mm j-loop
                            # being split, so the matmuls for the j-major
                            # case are emitted inside the STT loop instead.
                            jmajor = (
                                ob == N_OBLK - 1
                                and p_i == 2
                                and eg == N_EG - 1
                                and t == TCH - 1
                            )
                            if not jmajor:
                                for c in range(IC):
                                    lhsT = zt[:, c, bass.ts(t, P)]
                                    for j in range(EPG):
                                        nc.tensor.matmul(
                                            pks[j][:],
                                            lhsT=lhsT,
                                            rhs=wts[j][:, c, :],
                                            start=(c == 0),
                                            stop=(c == IC - 1),
                                        )
                            o_i = None
                            for j in range(EPG):
                                e = eg * EPG + j
                                gcol = g_all[:, t, e : e + 1]
                                if jmajor:
                                    for c in range(IC):
                                        nc.tensor.matmul(
                                            pks[j][:],
                                            lhsT=zt[:, c, bass.ts(t, P)],
                                            rhs=wts[j][:, c, :],
                                            start=(c == 0),
                                            stop=(c == IC - 1),
                                        )
                                if e == 0 and p_i == 2:
                                    # acc3 starts at g0*P3 - (acc1 + acc2)
                                    # (acc12 precomputed on Pool in pass 2),
                                    # so the last expert's STT below can
                                    # produce out_i directly
                                    nc.vector.scalar_tensor_tensor(
                                        out=acc[:, t, :],
                                        in0=pks[j][:],
                                        scalar=gcol,
                                        in1=acc12[:, t, :],
                                        op0=ALU.mult,
                                        op1=ALU.subtract,
                                    )
                                elif e == 0:
                                    nc.vector.tensor_scalar_mul(
                                        acc[:, t, :], pks[j][:], gcol
                                    )
                                elif e == E - 1 and p_i == 2:
                                    # final expert: emit out_i straight from
                                    # the STT -- no combine left to do
                                    o_i = opool.tile(
                                        [P, OBLK], dt.float32, name="o_i", tag="o_i"
                                    )
                                    if jmajor:
                                        # very last drain of the kernel:
                                        # split in halves so the first
                                        # half's store overlaps the second
                                        # half's STT
                                        hb = OBLK // 2
                                        for hh in range(2):
                                            osl = slice(hh * hb, (hh + 1) * hb)
                                            nc.vector.scalar_tensor_tensor(
                                                out=o_i[:, osl],
                                                in0=pks[j][:, osl],
                                                scalar=gcol,
                                                in1=acc[:, t, osl],
                                                op0=ALU.mult,
                                                op1=ALU.add,
                                            )
                                            dq = nc.scalar if hh == 0 else nc.sync
                                            dq.dma_start(
                                                out=d_oi[ob * TCH + t][:, osl],
                                                in_=o_i[:, osl],
                                            )
                                    else:
                                        nc.vector.scalar_tensor_tensor(
                                            out=o_i[:],
                                            in0=pks[j][:],
                                            scalar=gcol,
                                            in1=acc[:, t, :],
                                            op0=ALU.mult,
                                            op1=ALU.add,
                                        )
                                else:
                                    nc.vector.scalar_tensor_tensor(
                                        out=acc[:, t, :],
                                        in0=pks[j][:],
                                        scalar=gcol,
                                        in1=acc[:, t, :],
                                        op0=ALU.mult,
                                        op1=ALU.add,
                                    )

                            # combine + store inline once the last expert's
                            # contribution for this token chunk has landed
                            # (stores ride the ACT DMA queue, away from the
                            # weight stream)
                            # (combines ride the Pool engine, keeping DVE free
                            # for the STT stream; o_r only needs acc1/acc2 so
                            # it is combined and stored one pass early, on the
                            # otherwise-idle SP queue, so the output drain
                            # doesn't hold up the exit barrier)
                            if p_i == 1 and eg == N_EG - 1:
                                o_r = opool.tile(
                                    [P, OBLK], dt.float32, name="o_r", tag="o_r"
                                )
                                nc.gpsimd.tensor_sub(
                                    out=o_r[:], in0=acc1[:, t, :], in1=acc2[:, t, :]
                                )
                                nc.sync.dma_start(
                                    out=d_or[ob * TCH + t], in_=o_r[:]
                                )
                                nc.gpsimd.tensor_add(
                                    out=acc12[:, t, :],
                                    in0=acc1[:, t, :],
                                    in1=acc2[:, t, :],
                                )
                            if p_i == 2 and eg == N_EG - 1 and not jmajor:
                                nc.scalar.dma_start(
                                    out=d_oi[ob * TCH + t], in_=o_i[:]
                                )


    _dedupe_ldweights(nc)
    _hoist_boundary_waits(nc)
    nc.compile()
    return nc


def _dedupe_ldweights(nc):
    """Remove InstLdweights that reload the stationary already in the PE
    array.

    Tile legalization emits one InstLdweights per matmul even when EPG
    consecutive matmuls share the same stationary z chunk (our j-loop), so
    the PE instruction stream carries ~4x the needed weight loads.  The PE
    array is weight-stationary: a load persists until the next LDWEIGHTS, so
    an LDWEIGHTS whose source access pattern is identical to the previous
    one (with only matmuls / event-semaphores in between) is a no-op.
    Dropping them shrinks the PE instruction stream ~38%, reducing
    sequencer fetch stalls.

    LDWEIGHTS carrying sync_info are kept (their waits/updates must fire);
    any other PE instruction type conservatively invalidates the tracked
    signature.
    """
    for blk in nc.main_func.blocks:
        last_sig = None
        dead = []
        for inst in blk.instructions:
            if getattr(inst, "engine", None) != mybir.EngineType.PE:
                continue
            if isinstance(inst, mybir.InstLdweights):
                si = inst.sync_info
                has_sync = si is not None and (
                    len(si.on_wait) > 0 or len(si.on_update) > 0
                )
                sig = (
                    inst.concise(),
                    getattr(inst, "is_transpose", None),
                    getattr(inst, "perf_mode", None),
                )
                if sig == last_sig and not has_sync:
                    dead.append(inst)
                else:
                    last_sig = sig
            elif isinstance(inst, mybir.InstMatmult):
                if getattr(inst, "is_transpose", None):
                    last_sig = None
            elif isinstance(inst, mybir.InstEventSemaphore):
                pass
            else:
                last_sig = None
        for inst in dead:
            blk.instructions.remove(inst)


def _hoist_boundary_waits(nc):
    """Move semaphore waits off chain-start matmuls onto idle instruction
    slots a few matmuls earlier.

    At every (token-chunk x expert-group) boundary the 4 chain-start matmuls
    each carry a PSUM-reuse wait (DVE STT counter) or a weight-DMA wait.  On
    hardware the wait+LDWEIGHTS+issue sequence at a boundary does not fit
    inside the previous matmul's 213 ns streaming window, costing a ~432 ns
    bubble per boundary (~17 us total).  Hoisting the wait onto a clean
    LDWEIGHTS/MATMUL ~9-15 tensor-engine slots earlier keeps the boundary
    instructions wait-free so they pipeline.

    Safety: semaphores are monotonically increasing counters, so evaluating
    the same `>= threshold` earlier in program order only strengthens the
    ordering.  Deadlock is impossible because every producer of the hoisted
    sems (STTs of the group before last; weight DMAs prefetched a full
    expert-group ahead; the zs add at startup) depends only on instructions
    that precede the hoist target by at least one full 32-matmul group.
    The first 150 tensor instructions (warm-up, gates, first chains, where
    waits block for real) are left untouched.
    """
    blk = nc.main_func.blocks[1]
    te = [
        i
        for i in blk.instructions
        if getattr(i, "engine", None) == mybir.EngineType.PE
    ]
    taken = set()
    for ti, inst in enumerate(te):
        if ti <= 250 or not isinstance(inst, mybir.InstMatmult):
            continue
        si = inst.sync_info
        if si is None or not si.on_wait or not inst.start_tensor_calc:
            continue
        # Only DVE (PSUM-drain STT counter) and hardware-DMA-queue sems are
        # provably produced >=1 full group before the hoist target.  A wait
        # on the PE's own counter hoisted above its producers would
        # self-deadlock the engine; ACT waits belong to the gate phase.
        if not all(
            w.wait_mode == "sem-ge-imm"
            and (w.ant_name.startswith("DVE") or w.ant_name.startswith("DMAHW"))
            for w in si.on_wait
        ):
            continue
        host = None
        for k in range(9, 17):
            cand = te[ti - k]
            ci = cand.sync_info
            if id(cand) in taken or (ci is not None and ci.on_wait):
                continue
            if isinstance(cand, (mybir.InstLdweights, mybir.InstMatmult)):
                host = cand
                break
        if host is None:
            continue
        taken.add(id(host))
        hsi = host.sync_info
        if hsi is None:
            host.sync_info = mybir.SyncInfo(
                on_wait=list(si.on_wait), on_update=[]
            )
        else:
            hsi.on_wait = list(si.on_wait)
        si.on_wait = []


def _pack_w(WT_f32):
    """[E, in, out] fp32 -> bf16 [N_OBLK*E, P, IC, OBLK], fully contiguous
    per (ob, e) slice with element [obe, p, c, o] = W^T[e, c*P+p, ob*OBLK+o]."""
    w = WT_f32.astype(BF16).reshape(E, IC, P, N_OBLK, OBLK)
    return np.ascontiguousarray(w.transpose(3, 0, 2, 1, 4)).reshape(
        N_OBLK * E, P, IC, OBLK
    )


def _pack_z(z_f32_T):
    """[D, N_LOC] fp32 (already transposed) -> bf16 [P, IC, N_LOC]."""
    z = z_f32_T.astype(BF16).reshape(IC, P, N_LOC)
    return np.ascontiguousarray(z.transpose(1, 0, 2))


def kernel(z_real, z_imag, Wg, bg, Wr, Wi):
    global _BUILT, LAST_RESULTS
    assert z_real.shape == (N, D) and z_imag.shape == (N, D)

    if _BUILT is None:
        _BUILT = _build_module()
    nc = _BUILT

    # ---- host-side prep (layout + bf16 cast only)
    f32 = np.float32
    Wrf = np.asarray(Wr, f32)
    Wif = np.asarray(Wi, f32)
    wr_p = _pack_w(Wrf.transpose(0, 2, 1))
    wi_p = _pack_w(Wif.transpose(0, 2, 1))
    ws_p = _pack_w((Wrf + Wif).transpose(0, 2, 1))
    wg_p = np.ascontiguousarray(
        np.asarray(Wg, f32).T.astype(BF16).reshape(2 * IC, P, E).transpose(1, 0, 2)
    )
    bgc = np.ascontiguousarray(np.asarray(bg, f32).reshape(E, 1))

    zrT = np.asarray(z_real, f32).T                                 # [D, N]
    ziT = np.asarray(z_imag, f32).T

    in_maps = []
    for c in range(N_CORES):
        sl = slice(c * N_LOC, (c + 1) * N_LOC)
        in_maps.append(
            {
                "zr3": _pack_z(zrT[:, sl]),
                "zi3": _pack_z(ziT[:, sl]),
                "wr4": wr_p,
                "wi4": wi_p,
                "ws4": ws_p,
                "wg3": wg_p,
                "bgc": bgc,
            }
        )

    res = run_bass_kernel_spmd(
        nc, in_maps, core_ids=list(range(N_CORES)), trace=TRACE
    )
    LAST_RESULTS = res

    out = np.empty((2, N, D), dtype=np.float32)
    for c in range(N_CORES):
        sl = slice(c * N_LOC, (c + 1) * N_LOC)
        # [N_OBLK*TCH, P, OBLK] -> [N_LOC, D]
        o_r = res.results[c]["outr"].reshape(N_OBLK, TCH, P, OBLK)
        o_i = res.results[c]["outi"].reshape(N_OBLK, TCH, P, OBLK)
        out[0, sl] = o_r.transpose(1, 2, 0, 3).reshape(N_LOC, D)
        out[1, sl] = o_i.transpose(1, 2, 0, 3).reshape(N_LOC, D)
    return out



# revision 32
# speedup vs baseline: 1.0014x; 1.0014x over previous
"""ComplexMoE Trainium2 kernel.

Computes, for z_real/z_imag [N, D], gate weights Wg [E, 2D], bg [E], and
per-expert complex weights Wr/Wi [E, D, D]:

    gates = softmax(concat(z_r, z_i) @ Wg.T + bg)            [N, E]
    out_r = sum_e gates[:, e] * (z_r @ Wr_e.T - z_i @ Wi_e.T)
    out_i = sum_e gates[:, e] * (z_i @ Wr_e.T + z_r @ Wi_e.T)
    return stack([out_r, out_i])                             [2, N, D]

Strategy: data-parallel over tokens across 8 NeuronCores (1024 tokens each,
gate + expert weights replicated).  Per expert the complex matmul uses the
3-multiplication Karatsuba form:

    P1 = z_r @ Wr_e.T;  P2 = z_i @ Wi_e.T;  P3 = (z_r+z_i) @ (Wr_e+Wi_e).T
    out_r_e = P1 - P2;  out_i_e = P3 - P1 - P2

All 1024 tokens stay resident in SBUF (bf16), so each expert weight slice
streams from HBM exactly once.  Matmul operands are bf16 (same PE rate as
fp32r, half the HBM traffic, and FWL weight loads); PSUM accumulation and
the gated expert accumulators are fp32, so the only precision loss is bf16
operand rounding (measured ~4.1e-3 absmax rel on the reference
distribution, vs the 2e-2 gate).  Matmuls keep tokens on PSUM partitions
(stationary = z^T chunks, moving = W^T) so the per-token gate is a
per-partition scalar applied by fused scalar_tensor_tensor ops into three
SBUF accumulators; the final combines ride the Pool engine inline with the
last expert pass.

Perf notes (measured on trn2 via ntff profiles):
 - The PE streams 1 moving column/cycle at 2.4 GHz -> 3072 [128x128]x
   [128x512] matmuls = 665 us/core is the hard tensor-engine floor.
 - Each matmul normally pays a partially-exposed LDWEIGHTS (~45 ns);
   sharing one stationary z chunk across EPG=4 consecutive matmuls into 4
   PSUM banks (experts e..e+3 of the same product) removes nearly all of
   it (808 -> 675 us PE-active).
 - z loads, half of each weight group, and output stores ride the ACT
   hardware DMA queue; the rest rides the SP queue.  The two queues
   genuinely run in parallel (~350 GB/s each), halving the 8 MB critical
   startup prefix.
 - 48 throwaway warm-up matmuls keep the PE HAM clock-gate at K=8/8
   through the startup DMA window.
"""

import sys

try:
    import concourse.bass as bass  # noqa: F401
except ImportError:
    sys.path.insert(0, "/opt/trn_rl_repo")

import numpy as np
import ml_dtypes

import concourse.bass as bass
from concourse import bacc
import concourse.mybir as mybir
from concourse.tile import TileContext
from concourse.bass_utils import run_bass_kernel_spmd

dt = mybir.dt
BF16 = ml_dtypes.bfloat16

# ---------------------------------------------------------------- config
N_CORES = 8
N = 8192
D = 1024
E = 8
N_LOC = N // N_CORES            # tokens per core
P = 128                         # partitions
IC = D // P                     # 8 contraction chunks per z tensor
OBLK = 512                      # output-feature block (one PSUM bank)
N_OBLK = D // OBLK              # 2
TCH = N_LOC // P                # 8 token chunks of 128
GTG = 2                         # gate token groups (PSUM free-size limit)
GT = N_LOC // GTG               # 512 tokens per gate group

EPG = 4                         # experts per group (stationary z chunk shared
                                # across EPG consecutive matmuls -> 1 LDWEIGHTS
                                # per EPG matmuls if codegen elides duplicates)
N_EG = E // EPG                 # 2

TRACE = False                   # set by test harness to capture HW timing
LAST_RESULTS = None             # BassKernelResults of the last run

_BUILT = None


def _build_module():
    nc = bacc.Bacc("TRN2", target_bir_lowering=False, debug=False)

    # Drop the (unused) software-DGE queue declaration: this kernel only
    # issues hardware-DGE transfers (sync/scalar), and each declared ring
    # costs a completion semaphore that the NEFF epilogue clears serially
    # (~110 ns x 16 rings on every engine at exit).
    nc.m.queues = [q for q in nc.m.queues if getattr(q, "is_HWDGE", False)]

    bf = dt.bfloat16
    d_zr = nc.dram_tensor("zr3", [P, IC, N_LOC], bf, kind="ExternalInput").ap()
    d_zi = nc.dram_tensor("zi3", [P, IC, N_LOC], bf, kind="ExternalInput").ap()
    WSHP = [N_OBLK * E, P, IC, OBLK]
    d_wr = nc.dram_tensor("wr4", WSHP, bf, kind="ExternalInput").ap()
    d_wi = nc.dram_tensor("wi4", WSHP, bf, kind="ExternalInput").ap()
    d_ws = nc.dram_tensor("ws4", WSHP, bf, kind="ExternalInput").ap()
    d_wg = nc.dram_tensor("wg3", [P, 2 * IC, E], bf, kind="ExternalInput").ap()
    d_bg = nc.dram_tensor("bgc", [E, 1], dt.float32, kind="ExternalInput").ap()
    OSHP = [N_OBLK * TCH, P, OBLK]
    d_or = nc.dram_tensor("outr", OSHP, dt.float32, kind="ExternalOutput").ap()
    d_oi = nc.dram_tensor("outi", OSHP, dt.float32, kind="ExternalOutput").ap()

    AF = mybir.ActivationFunctionType
    ALU = mybir.AluOpType

    with TileContext(nc, trace_sim=False) as tc:
        with (
            tc.tile_pool(name="cst", bufs=1) as cpool,
            tc.tile_pool(name="zres", bufs=1) as zpool,
            tc.tile_pool(name="wmov", bufs=2) as wpool,
            tc.tile_pool(name="accs", bufs=1) as apool,
            tc.tile_pool(name="gate", bufs=2) as gpool,
            tc.tile_pool(name="outs", bufs=2) as opool,
            tc.tile_pool(name="ps_main", bufs=2, space="PSUM") as pspool,
        ):
            # ---- constants: gate weights + bias
            wg = cpool.tile([P, 2 * IC, E], bf, name="wg")
            nc.sync.dma_start(out=wg[:], in_=d_wg)
            bgc = cpool.tile([E, 1], dt.float32, name="bgc")
            nc.sync.dma_start(out=bgc[:], in_=d_bg)
            ident = cpool.tile([E, E], dt.float32, name="ident")
            from concourse.masks import make_identity

            make_identity(nc, ident[:])

            # ---- resident z^T tensors (full 1024 tokens): [P, IC, N_LOC]
            # z rides the ACT hardware DMA queue so it loads in parallel with
            # the first weight tiles on the SP queue; zs = zr + zi is computed
            # on-device (DVE) -- it is not needed until the third product pass.
            zr = zpool.tile([P, IC, N_LOC], bf, name="zr", tag="zr")
            zi = zpool.tile([P, IC, N_LOC], bf, name="zi", tag="zi")
            zs = zpool.tile([P, IC, N_LOC], bf, name="zs", tag="zs")
            nc.sync.dma_start(out=zr[:], in_=d_zr)
            nc.scalar.dma_start(out=zi[:], in_=d_zi)

            # ---- PE warm-up: throwaway matmuls on the (tiny, already
            # loaded) gate weights keep the PE HAM clock-gate engaged
            # through the z/weight DMA window, so the gate matmuls and
            # first expert chains run at full clock
            for wu in range(48):
                wups = pspool.tile([E, P], dt.float32, name="wup", tag="pj3")
                nc.tensor.matmul(
                    wups[:], lhsT=wg[:, 0, :], rhs=wg[:, :, :], start=True, stop=True
                )

            # ---- gates: logits^T [E, GT] in PSUM, softmax via ACT + DVE
            # (gate PSUM tiles borrow the main pool's expert tags pj0/pj1 --
            # they are drained long before the main loop's allocations rotate
            # around to the same slots)
            g_all = gpool.tile([P, TCH, E], dt.float32, name="g_all", tag="g_all")
            for tg in range(GTG):
                tsl = bass.ts(tg, GT)
                lgT = pspool.tile([E, GT], dt.float32, name="lgT", tag="pj0")
                for c in range(2 * IC):
                    zsrc = zr if c < IC else zi
                    nc.tensor.matmul(
                        lgT[:],
                        lhsT=wg[:, c, :],
                        rhs=zsrc[:, c % IC, tsl],
                        start=(c == 0),
                        stop=(c == 2 * IC - 1),
                    )
                uT = gpool.tile([E, GT], dt.float32, name="uT", tag="uT")
                nc.scalar.activation(uT[:], lgT[:], AF.Exp, bias=bgc[:])
                for t in range(GT // P):
                    gt = tg * (GT // P) + t
                    tp = pspool.tile([P, E], dt.float32, name="tp", tag="pj1")
                    nc.tensor.transpose(
                        tp[:], in_=uT[:, bass.ts(t, P)], identity=ident[:]
                    )
                    s = gpool.tile([P, 1], dt.float32, name="s", tag="s")
                    nc.vector.tensor_reduce(
                        s[:], tp[:], axis=mybir.AxisListType.X, op=ALU.add
                    )
                    r = gpool.tile([P, 1], dt.float32, name="r", tag="r")
                    nc.vector.reciprocal(r[:], s[:])
                    nc.vector.tensor_scalar_mul(g_all[:, gt, :], tp[:], r[:])

            # zs = zr + zi on DVE (after the gate DVE ops so the gates are
            # ready for the first STT; zs itself is needed much later).  The
            # early-stream cadence is weight-DMA-bound either way, so this
            # placement is not on the critical path.
            nc.vector.tensor_add(out=zs[:], in0=zr[:], in1=zi[:])

            # ---- main loop: weights stream once; per-product passes with
            # EPG experts sharing each stationary z chunk (one LDWEIGHTS
            # feeds EPG matmuls into EPG PSUM banks)
            for ob in range(N_OBLK):
                acc1 = apool.tile([P, TCH, OBLK], dt.float32, name="acc1", tag="acc1")
                acc2 = apool.tile([P, TCH, OBLK], dt.float32, name="acc2", tag="acc2")
                acc3 = apool.tile([P, TCH, OBLK], dt.float32, name="acc3", tag="acc3")
                acc12 = apool.tile(
                    [P, TCH, OBLK], dt.float32, name="acc12", tag="acc12"
                )
                for p_i, (zt, d_w, acc) in enumerate(
                    ((zr, d_wr, acc1), (zi, d_wi, acc2), (zs, d_ws, acc3))
                ):
                    for eg in range(N_EG):
                        wts = []
                        for j in range(EPG):
                            e = eg * EPG + j
                            wt = wpool.tile(
                                [P, IC, OBLK], bf, name=f"w{j}", tag=f"w{j}"
                            )
                            dma_eng = nc.sync if j < EPG // 2 else nc.scalar
                            dma_eng.dma_start(out=wt[:], in_=d_w[ob * E + e])
                            wts.append(wt)

                        for t in range(TCH):
                            pks = [
                                pspool.tile(
                                    [P, OBLK], dt.float32, name=f"pj{j}", tag=f"pj{j}"
                                )
                                for j in range(EPG)
                            ]
                            # The very last chain group runs expert-major
                            # (full c-chain per expert) so each bank's STT
                            # can start as soon as that expert's chain ends;
                            # only the final expert's STT + store trail the
                            # last matmul (~0.75 us) instead of the whole
                            # 4-STT chain (~3 us).  Interleaving the STT
                            # emission per-j below relies on the mm j-loop
                            # being split, so the matmuls for the j-major
                            # case are emitted inside the STT loop instead.
                            jmajor = (
                                ob == N_OBLK - 1
                                and p_i == 2
                                and eg == N_EG - 1
                                and t == TCH - 1
                            )
                            if not jmajor:
                                for c in range(IC):
                                    lhsT = zt[:, c, bass.ts(t, P)]
                                    for j in range(EPG):
                                        nc.tensor.matmul(
                                            pks[j][:],
                                            lhsT=lhsT,
                                            rhs=wts[j][:, c, :],
                                            start=(c == 0),
                                            stop=(c == IC - 1),
                                        )
                            o_i = None
                            for j in range(EPG):
                                e = eg * EPG + j
                                gcol = g_all[:, t, e : e + 1]
                                if jmajor:
                                    for c in range(IC):
                                        nc.tensor.matmul(
                                            pks[j][:],
                                            lhsT=zt[:, c, bass.ts(t, P)],
                                            rhs=wts[j][:, c, :],
                                            start=(c == 0),
                                            stop=(c == IC - 1),
                                        )
                                if e == 0 and p_i == 2:
                                    # acc3 starts at g0*P3 - (acc1 + acc2)
                                    # (acc12 precomputed on Pool in pass 2),
                                    # so the last expert's STT below can
                                    # produce out_i directly
                                    nc.vector.scalar_tensor_tensor(
                                        out=acc[:, t, :],
                                        in0=pks[j][:],
                                        scalar=gcol,
                                        in1=acc12[:, t, :],
                                        op0=ALU.mult,
                                        op1=ALU.subtract,
                                    )
                                elif e == 0:
                                    nc.vector.tensor_scalar_mul(
                                        acc[:, t, :], pks[j][:], gcol
                                    )
                                elif e == E - 1 and p_i == 2:
                                    # final expert: emit out_i straight from
                                    # the STT -- no combine left to do
                                    o_i = opool.tile(
                                        [P, OBLK], dt.float32, name="o_i", tag="o_i"
                                    )
                                    if jmajor:
                                        # very last drain of the kernel:
                                        # split in halves so the first
                                        # half's store overlaps the second
                                        # half's STT
                                        hb = OBLK // 2
                                        for hh in range(2):
                                            osl = slice(hh * hb, (hh + 1) * hb)
                                            nc.vector.scalar_tensor_tensor(
                                                out=o_i[:, osl],
                                                in0=pks[j][:, osl],
                                                scalar=gcol,
                                                in1=acc[:, t, osl],
                                                op0=ALU.mult,
                                                op1=ALU.add,
                                            )
                                            dq = nc.scalar if hh == 0 else nc.sync
                                            dq.dma_start(
                                                out=d_oi[ob * TCH + t][:, osl],
                                                in_=o_i[:, osl],
                                            )
                                    else:
                                        nc.vector.scalar_tensor_tensor(
                                            out=o_i[:],
                                            in0=pks[j][:],
                                            scalar=gcol,
                                            in1=acc[:, t, :],
                                            op0=ALU.mult,
                                            op1=ALU.add,
                                        )
                                else:
                                    nc.vector.scalar_tensor_tensor(
                                        out=acc[:, t, :],
                                        in0=pks[j][:],
                                        scalar=gcol,
                                        in1=acc[:, t, :],
                                        op0=ALU.mult,
                                        op1=ALU.add,
                                    )

                            # combine + store inline once the last expert's
                            # contribution for this token chunk has landed
                            # (stores ride the ACT DMA queue, away from the
                            # weight stream)
                            # (combines ride the Pool engine, keeping DVE free
                            # for the STT stream; o_r only needs acc1/acc2 so
                            # it is combined and stored one pass early, on the
                            # otherwise-idle SP queue, so the output drain
                            # doesn't hold up the exit barrier)
                            if p_i == 1 and eg == N_EG - 1:
                                o_r = opool.tile(
                                    [P, OBLK], dt.float32, name="o_r", tag="o_r"
                                )
                                nc.gpsimd.tensor_sub(
                                    out=o_r[:], in0=acc1[:, t, :], in1=acc2[:, t, :]
                                )
                                nc.sync.dma_start(
                                    out=d_or[ob * TCH + t], in_=o_r[:]
                                )
                                nc.gpsimd.tensor_add(
                                    out=acc12[:, t, :],
                                    in0=acc1[:, t, :],
                                    in1=acc2[:, t, :],
                                )
                            if p_i == 2 and eg == N_EG - 1 and not jmajor:
                                nc.scalar.dma_start(
                                    out=d_oi[ob * TCH + t], in_=o_i[:]
                                )


    _dedupe_ldweights(nc)
    _hoist_boundary_waits(nc)
    nc.compile()
    return nc


def _dedupe_ldweights(nc):
    """Remove InstLdweights that reload the stationary already in the PE
    array.

    Tile legalization emits one InstLdweights per matmul even when EPG
    consecutive matmuls share the same stationary z chunk (our j-loop), so
    the PE instruction stream carries ~4x the needed weight loads.  The PE
    array is weight-stationary: a load persists until the next LDWEIGHTS, so
    an LDWEIGHTS whose source access pattern is identical to the previous
    one (with only matmuls / event-semaphores in between) is a no-op.
    Dropping them shrinks the PE instruction stream ~38%, reducing
    sequencer fetch stalls.

    LDWEIGHTS carrying sync_info are kept (their waits/updates must fire);
    any other PE instruction type conservatively invalidates the tracked
    signature.
    """
    for blk in nc.main_func.blocks:
        last_sig = None
        dead = []
        for inst in blk.instructions:
            if getattr(inst, "engine", None) != mybir.EngineType.PE:
                continue
            if isinstance(inst, mybir.InstLdweights):
                si = inst.sync_info
                has_sync = si is not None and (
                    len(si.on_wait) > 0 or len(si.on_update) > 0
                )
                sig = (
                    inst.concise(),
                    getattr(inst, "is_transpose", None),
                    getattr(inst, "perf_mode", None),
                )
                if sig == last_sig and not has_sync:
                    dead.append(inst)
                else:
                    last_sig = sig
            elif isinstance(inst, mybir.InstMatmult):
                if getattr(inst, "is_transpose", None):
                    last_sig = None
            elif isinstance(inst, mybir.InstEventSemaphore):
                pass
            else:
                last_sig = None
        for inst in dead:
            blk.instructions.remove(inst)


def _hoist_boundary_waits(nc):
    """Move semaphore waits off chain-start matmuls onto idle instruction
    slots a few matmuls earlier.

    At every (token-chunk x expert-group) boundary the 4 chain-start matmuls
    each carry a PSUM-reuse wait (DVE STT counter) or a weight-DMA wait.  On
    hardware the wait+LDWEIGHTS+issue sequence at a boundary does not fit
    inside the previous matmul's 213 ns streaming window, costing a ~432 ns
    bubble per boundary (~17 us total).  Hoisting the wait onto a clean
    LDWEIGHTS/MATMUL ~9-15 tensor-engine slots earlier keeps the boundary
    instructions wait-free so they pipeline.

    Safety: semaphores are monotonically increasing counters, so evaluating
    the same `>= threshold` earlier in program order only strengthens the
    ordering.  Deadlock is impossible because every producer of the hoisted
    sems (STTs of the group before last; weight DMAs prefetched a full
    expert-group ahead; the zs add at startup) depends only on instructions
    that precede the hoist target by at least one full 32-matmul group.
    The first 150 tensor instructions (warm-up, gates, first chains, where
    waits block for real) are left untouched.
    """
    blk = nc.main_func.blocks[1]
    te = [
        i
        for i in blk.instructions
        if getattr(i, "engine", None) == mybir.EngineType.PE
    ]
    taken = set()
    for ti, inst in enumerate(te):
        if ti <= 250 or not isinstance(inst, mybir.InstMatmult):
            continue
        si = inst.sync_info
        if si is None or not si.on_wait or not inst.start_tensor_calc:
            continue
        # Only DVE (PSUM-drain STT counter) and hardware-DMA-queue sems are
        # provably produced >=1 full group before the hoist target.  A wait
        # on the PE's own counter hoisted above its producers would
        # self-deadlock the engine; ACT waits belong to the gate phase.
        if not all(
            w.wait_mode == "sem-ge-imm"
            and (w.ant_name.startswith("DVE") or w.ant_name.startswith("DMAHW"))
            for w in si.on_wait
        ):
            continue
        host = None
        for k in range(9, 17):
            cand = te[ti - k]
            ci = cand.sync_info
            if id(cand) in taken or (ci is not None and ci.on_wait):
                continue
            if isinstance(cand, (mybir.InstLdweights, mybir.InstMatmult)):
                host = cand
                break
        if host is None:
            continue
        taken.add(id(host))
        hsi = host.sync_info
        if hsi is None:
            host.sync_info = mybir.SyncInfo(
                on_wait=list(si.on_wait), on_update=[]
            )
        else:
            hsi.on_wait = list(si.on_wait)
        si.on_wait = []


def _pack_w(WT_f32):
    """[E, in, out] fp32 -> bf16 [N_OBLK*E, P, IC, OBLK], fully contiguous
    per (ob, e) slice with element [obe, p, c, o] = W^T[e, c*P+p, ob*OBLK+o]."""
    w = WT_f32.astype(BF16).reshape(E, IC, P, N_OBLK, OBLK)
    return np.ascontiguousarray(w.transpose(3, 0, 2, 1, 4)).reshape(
        N_OBLK * E, P, IC, OBLK
    )


def _pack_z(z_f32_T):
    """[D, N_LOC] fp32 (already transposed) -> bf16 [P, IC, N_LOC]."""
    z = z_f32_T.astype(BF16).reshape(IC, P, N_LOC)
    return np.ascontiguousarray(z.transpose(1, 0, 2))


def kernel(z_real, z_imag, Wg, bg, Wr, Wi):
    global _BUILT, LAST_RESULTS
    assert z_real.shape == (N, D) and z_imag.shape == (N, D)

    if _BUILT is None:
        _BUILT = _build_module()
    nc = _BUILT

    # ---- host-side prep (layout + bf16 cast only)
    f32 = np.float32
    Wrf = np.asarray(Wr, f32)
    Wif = np.asarray(Wi, f32)
    wr_p = _pack_w(Wrf.transpose(0, 2, 1))
    wi_p = _pack_w(Wif.transpose(0, 2, 1))
    ws_p = _pack_w((Wrf + Wif).transpose(0, 2, 1))
    wg_p = np.ascontiguousarray(
        np.asarray(Wg, f32).T.astype(BF16).reshape(2 * IC, P, E).transpose(1, 0, 2)
    )
    bgc = np.ascontiguousarray(np.asarray(bg, f32).reshape(E, 1))

    zrT = np.asarray(z_real, f32).T                                 # [D, N]
    ziT = np.asarray(z_imag, f32).T

    in_maps = []
    for c in range(N_CORES):
        sl = slice(c * N_LOC, (c + 1) * N_LOC)
        in_maps.append(
            {
                "zr3": _pack_z(zrT[:, sl]),
                "zi3": _pack_z(ziT[:, sl]),
                "wr4": wr_p,
                "wi4": wi_p,
                "ws4": ws_p,
                "wg3": wg_p,
                "bgc": bgc,
            }
        )

    res = run_bass_kernel_spmd(
        nc, in_maps, core_ids=list(range(N_CORES)), trace=TRACE
    )
    LAST_RESULTS = res

    out = np.empty((2, N, D), dtype=np.float32)
    for c in range(N_CORES):
        sl = slice(c * N_LOC, (c + 1) * N_LOC)
        # [N_OBLK*TCH, P, OBLK] -> [N_LOC, D]
        o_r = res.results[c]["outr"].reshape(N_OBLK, TCH, P, OBLK)
        o_i = res.results[c]["outi"].reshape(N_OBLK, TCH, P, OBLK)
        out[0, sl] = o_r.transpose(1, 2, 0, 3).reshape(N_LOC, D)
        out[1, sl] = o_i.transpose(1, 2, 0, 3).reshape(N_LOC, D)
    return out

